# revision 11
# baseline (speedup 1.0000x reference)
"""Trainium2 Bass kernel for nn_CropPrompter.

Fused resize+crop bilinear sampling of video clips:
  x[8,3,16,512,512] --(per-clip crop geometry from cam_views/resize/offsets)-->
  out[8,3,16,224,224]

Strategy (pure data parallel, 1 clip per NeuronCore, 8 cores):
  * Host computes, in float32 (bit-matching the reference math), the source
    coordinates and bilinear weights per clip, and packs them as two sparse
    interpolation matrices Ry / Rx (2 nonzeros per output index).  The source
    window [H0:H0+2*Pr) x [W0:W0+Wx) is computed from the runtime
    resize/offset values (max over the views referenced by cam_views); the
    device program is compiled per geometry and cached.
  * Input DMA: row pairs packed as one contiguous (512+Wx)-float DRAM run
    per (pair, frame) -- sub-row runs are descriptor-bound (~170 GB/s) while
    3 KB runs stream at >450 GB/s -- cast to bf16 in flight on the SWDGE
    (gpsimd) path, which keeps the HWDGE rings free for the output stores.
  * Device, per frame: out = Ry^T @ win @ Rx via two TensorE matmul stages:
      A^T[w,i] = sum_h win[h,w] * Ry[h,i]    (lhsT=win k-tile, rhs=Ry)
      out[i,j] = sum_w A^T[w,i] * Rx[w,j]    (lhsT=A^T, rhs=Rx)
    all in bf16 (1 cycle/row at any free size, and LDWEIGHTS takes the
    fast-weight-load path, which mixed fp32 streams would disable).
  * Ry columns are permuted (even i -> 0:112, odd i -> 128:240, zero pads)
    so stage-2 M-tiles are 128-wide and the output store writes contiguous
    row-pair runs.
"""

import numpy as np

CROP = 224
H = 512
RESIZE_MAX = 1024
PAD_I = 256  # permuted crop-row space: even i at 0:112, odd i at 128:240

_PROGRAMS = {}
TRACE = False
LAST_RESULTS = None


def _coords(off, rb):
    """Replicates reference._coords in numpy float32, op-for-op."""
    i = np.arange(CROP, dtype=np.float32)
    src = (np.float32(off) + i + np.float32(0.5)) * (np.float32(H) / np.float32(rb)) - np.float32(0.5)
    src = np.maximum(src, np.float32(0.0))
    i0 = np.clip(np.floor(src).astype(np.int32), 0, H - 1)
    i1 = np.minimum(i0 + 1, H - 1)
    w = src - i0.astype(np.float32)
    return i0, i1, w


def _reference_cpu(x, cam_views, resize, y_offset, x_offset):
    """Numpy fallback for geometries outside the compiled envelope."""
    r = np.floor(np.clip(resize, np.float32(H), np.float32(RESIZE_MAX)))
    yo = np.floor(np.clip(y_offset, np.float32(0.0), r - np.float32(CROP)))
    xo = np.floor(np.clip(x_offset, np.float32(0.0), r - np.float32(CROP)))
    out = np.empty((x.shape[0], 3, 16, CROP, CROP), dtype=np.float32)
    for b in range(x.shape[0]):
        v = int(cam_views[b])
        y0, y1, wy = _coords(yo[v], r[v])
        x0, x1, wx = _coords(xo[v], r[v])
        clip = x[b]
        rows = clip[:, :, y0, :] * (1.0 - wy)[:, None] + clip[:, :, y1, :] * wy[:, None]
        out[b] = rows[:, :, :, x0] * (1.0 - wx) + rows[:, :, :, x1] * wx
    return out


def _split_multi_waits(nc):
    """Walrus allows only one semaphore wait per instruction; hoist extra
    waits onto standalone EventSemaphore instructions on the same engine."""
    from concourse import mybir

    n = 0
    for fn in nc.m.functions:
        for bb in fn.blocks:
            out = []
            changed = False
            for inst in bb.instructions:
                si = getattr(inst, "sync_info", None)
                waits = list(si.on_wait) if si is not None and si.on_wait else []
                if len(waits) > 1:
                    for k, w in enumerate(waits[:-1]):
                        out.append(
                            mybir.InstEventSemaphore(
                                name=f"{inst.name}-w{k}",
                                ins=[],
                                outs=[],
                                engine=inst.engine,
                                sync_info=mybir.SyncInfo(on_wait=[w], on_update=[]),
                            )
                        )
                        n += 1
                    inst.sync_info = mybir.SyncInfo(
                        on_wait=[waits[-1]], on_update=list(si.on_update or [])
                    )
                    changed = True
                out.append(inst)
            if changed:
                bb.instructions = out
    return n


def _build_program(Pr, Wx, H0, W0):
    """Pr row pairs starting at row H0; Wx window cols starting at W0."""
    from concourse import bass, mybir, tile

    f32 = mybir.dt.float32
    f32r = mybir.dt.float32r
    bf16 = mybir.dt.bfloat16

    M0 = min(Wx, 128)
    M1 = Wx - M0  # cols in second w block (0 if Wx <= 128)
    wms = [(0, M0)] + ([(M0, M1)] if M1 else [])
    V = 512 + Wx  # packed pair-run length: row 2p cols [W0:512], 2p+1 [0:W0+Wx]

    nc = bass.Bass()
    xc = nc.dram_tensor("xc", [3, 16, H, H], f32r, kind="ExternalInput")
    ry = nc.dram_tensor("ry", [Pr, 2, PAD_I], f32r, kind="ExternalInput")
    rx = nc.dram_tensor("rx", [128, 2, CROP], bf16, kind="ExternalInput")
    out = nc.dram_tensor("out", [3, 16, CROP, CROP], f32, kind="ExternalOutput")

    with tile.TileContext(nc) as tc:
        with (
            tc.tile_pool(name="const", bufs=1) as constp,
            tc.tile_pool(name="xin", bufs=2) as xinp,
            tc.tile_pool(name="atp", bufs=4) as atp,
            tc.tile_pool(name="otp", bufs=2) as otp,
            tc.tile_pool(name="psa", bufs=4, space="PSUM") as psap,
            tc.tile_pool(name="pso", bufs=3, space="PSUM") as psop,
        ):
            ryt = constp.tile([Pr, 2, PAD_I], f32r)
            rxt = constp.tile([128, 2, CROP], bf16)
            nc.sync.dma_start(out=ryt[:], in_=ry[:])
            nc.sync.dma_start(out=rxt[:], in_=rx[:])

            xw_c = {}

            def issue_in(c):
                # window tile: [pair, t, v] holding rows (2p, 2p+1) of each
                # frame as one (512+Wx)-float contiguous DRAM run starting at
                # (row H0+2p, col W0): window cols of row 2p at v in [0,Wx),
                # of row 2p+1 at v in [512, 512+Wx).  Cast fp32->bf16 in
                # flight (SWDGE).
                xw_c[c] = xinp.tile([Pr, 16, V], f32r, name="xw", tag="xw")
                src_pairs = xc[c, :, H0 : H0 + 2 * Pr, :].rearrange(
                    "t (pr r) w -> pr t (r w)", pr=Pr, r=2
                )
                steps = (
                    (slice(0, 4), slice(4, 8), slice(8, 12), slice(12, 16))
                    if c == 0
                    else (slice(0, 8), slice(8, 16))
                )
                for th in steps:
                    nc.sync.dma_start(
                        out=xw_c[c][:, th, :],
                        in_=src_pairs[:, th, W0 : W0 + V],
                    )

            issue_in(0)
            issue_in(1)

            for c in range(3):
                if c + 1 < 3 and c + 1 not in xw_c:
                    issue_in(c + 1)
                xw = xw_c[c]

                ot = None
                psa_t = {}

                def issue_mm1(t):
                    psa_t[t] = psap.tile(
                        [128, len(wms), PAD_I], f32, name="psa", tag="psa"
                    )
                    psa = psa_t[t]
                    for mi, (w0, mm) in enumerate(wms):
                        for j in range(2):  # row parity k-tiles
                            nc.tensor.matmul(
                                psa[0:mm, mi, :],
                                lhsT=xw[:, t, j * 512 + w0 : j * 512 + w0 + mm],
                                rhs=ryt[:, j, :],
                                start=(j == 0),
                                stop=(j == 1),
                            )

                def stage2(t):
                    psa = psa_t.pop(t)
                    at = atp.tile([128, len(wms), PAD_I], bf16, name="at", tag="at")
                    for mi, (w0, mm) in enumerate(wms):
                        nc.vector.tensor_copy(at[0:mm, mi, :], psa[0:mm, mi, :])
                    pso = psop.tile([128, 2, CROP], f32, name="pso", tag="pso")
                    for m2 in range(2):
                        for qi, (w0, mm) in enumerate(wms):
                            nc.tensor.matmul(
                                pso[:, m2, :],
                                lhsT=at[0:mm, qi, m2 * 128 : m2 * 128 + 128],
                                rhs=rxt[0:mm, qi, :],
                                start=(qi == 0),
                                stop=(qi == len(wms) - 1),
                            )
                    nc.scalar.copy(out=ot[:, t % 4, :, :], in_=pso[0:112, :, 0:CROP])
                    if t % 4 == 3:
                        # store quarter-channel on the ACT HWDGE ring as
                        # row-pair runs: out rows (2p, 2p+1) are one
                        # contiguous 1792 B write per (pair, frame)
                        th = slice(t - 3, t + 1)
                        nc.scalar.dma_start(
                            out=out[c, th, :, :].rearrange(
                                "t (p r) j -> p t (r j)", p=112, r=2
                            ),
                            in_=ot[:, :, :, :].rearrange("p t r j -> p t (r j)"),
                        )

                for g in range(8):  # 2-frame groups, software-pipelined
                    if g % 2 == 0:
                        ot = otp.tile([112, 4, 2, CROP], f32, name="ot", tag="ot")
                    if g == 0:
                        issue_mm1(0)
                        issue_mm1(1)
                    for t in (2 * g + 2, 2 * g + 3):
                        if t < 16:
                            issue_mm1(t)
                    stage2(2 * g)
                    stage2(2 * g + 1)
    _split_multi_waits(nc)
    return nc


def kernel(x, cam_views, resize, y_offset, x_offset):
    global LAST_RESULTS
    import ml_dtypes
    from concourse.bass_utils import run_bass_kernel_spmd

    x = np.ascontiguousarray(np.asarray(x), dtype=np.float32)
    cam_views = np.asarray(cam_views)
    resize = np.asarray(resize, dtype=np.float32)
    y_offset = np.asarray(y_offset, dtype=np.float32)
    x_offset = np.asarray(x_offset, dtype=np.float32)

    B = x.shape[0]
    assert x.shape == (8, 3, 16, H, H), x.shape

    # reference's clamp/floor in float32
    r = np.floor(np.clip(resize, np.float32(H), np.float32(RESIZE_MAX)))
    yo = np.floor(np.clip(y_offset, np.float32(0.0), r - np.float32(CROP)))
    xo = np.floor(np.clip(x_offset, np.float32(0.0), r - np.float32(CROP)))

    views = sorted(set(int(v) for v in cam_views))
    ycoords = {v: _coords(yo[v], r[v]) for v in views}
    xcoords = {v: _coords(xo[v], r[v]) for v in views}
    H0 = int(min(ycoords[v][0].min() for v in views))
    W0 = int(min(xcoords[v][0].min() for v in views))
    Wy = int(max(ycoords[v][1].max() for v in views)) + 1 - H0
    Wx = int(max(xcoords[v][1].max() for v in views)) + 1 - W0
    Wx = (Wx + 7) & ~7
    Pr = (Wy + 1) // 2

    if not (Wx <= 256 and Pr <= 128 and H0 + 2 * Pr <= H and W0 + 512 + Wx <= 1024):
        # geometry outside the compiled envelope (cannot happen for the
        # spec's randint(0,32) offsets) -- compute on host instead
        return _reference_cpu(x, cam_views, resize, y_offset, x_offset)

    # pack interpolation matrices: ry [Pr,2,PAD_I] (row h = H0+2p+j),
    # columns permuted so stage-2 M-tiles are 128-wide; rx [128,2,224]
    # (w k-tiles of the window col space)
    idx = np.arange(CROP)
    pidx = np.where(idx % 2 == 0, idx // 2, 128 + idx // 2)
    ry_v, rx_v = {}, {}
    for v in views:
        y0, y1, wy = ycoords[v]
        m = np.zeros((2 * Pr, PAD_I), dtype=np.float32)
        np.add.at(m, (y0 - H0, pidx), np.float32(1.0) - wy)
        np.add.at(m, (y1 - H0, pidx), wy)
        ry_v[v] = np.ascontiguousarray(m.reshape(Pr, 2, PAD_I))  # fp32 bits

        x0, x1, wx = xcoords[v]
        m = np.zeros((256, CROP), dtype=np.float32)
        np.add.at(m, (x0 - W0, idx), np.float32(1.0) - wx)
        np.add.at(m, (x1 - W0, idx), wx)
        p = np.zeros((128, 2, CROP), dtype=np.float32)
        p[:, 0, :] = m[0:128]
        p[: max(Wx - 128, 0), 1, :] = m[128 : max(Wx, 128)]
        rx_v[v] = np.ascontiguousarray(p.astype(ml_dtypes.bfloat16))

    key = (Pr, Wx, H0, W0)
    if key not in _PROGRAMS:
        _PROGRAMS.clear()
        _PROGRAMS[key] = _build_program(Pr, Wx, H0, W0)
    prog = _PROGRAMS[key]

    in_maps = []
    for b in range(B):
        v = int(cam_views[b])
        in_maps.append(
            {"xc": np.ascontiguousarray(x[b]), "ry": ry_v[v], "rx": rx_v[v]}
        )

    res = run_bass_kernel_spmd(prog, in_maps, list(range(B)), trace=TRACE)
    LAST_RESULTS = res
    return np.stack([res.results[b]["out"] for b in range(B)], axis=0)
